# revision 6
# baseline (speedup 1.0000x reference)
"""Trainium2 Bass kernel for nn_CoAttention_Simple (B=8,C=4,T=1024,D=512).

Sharding: data-parallel over B across the 8 NeuronCores (core i handles
batch b=i). Each core runs the full per-batch pipeline:

  in_proj (q/k/v) -> 3x gated-linear MLP -> 8-head attention ->
  out_proj -> double residual + LayerNorm -> mean over C

All matmuls in bf16 (fp32 PSUM accumulation); softmax/layernorm math in
fp32. Activations flow feature-major ("transposed", (feat, t)) through
the gated chains so no on-device transposes are needed; the v-chain's
last matmul swaps lhsT/rhs roles to emit v in (t, feat) layout, which is
exactly what the attention AV matmul needs. Attention computes S^T=(s,t)
so the softmax denominator is a ones-matmul column sum (logits are ~1e-1
so exp needs no max subtraction).
"""

import sys

for _p in ("/opt/trn_rl_repo", "/root/.axon_site/_ro/trn_rl_repo"):
    if _p not in sys.path:
        sys.path.insert(0, _p)

import numpy as np
import ml_dtypes

B, C, T, D = 8, 4, 1024, 512
E = 512
A = C * E          # 2048
H = 8
HD = A // H        # 256
EPS = 1e-5
P = 128
BF = ml_dtypes.bfloat16

_prog_cache = {}


def _build_program():
    import concourse.mybir as mybir
    import concourse.tile as tile
    from concourse import bacc

    dt = mybir.dt
    f32, bf16 = dt.float32, dt.bfloat16
    AF = mybir.ActivationFunctionType
    OP = mybir.AluOpType

    nc = bacc.Bacc(None, target_bir_lowering=False)

    def inp(name, shape, dty=bf16):
        return nc.declare_dram_parameter(name, list(shape), dty, isOutput=False)

    # ---- external inputs (host pre-transposed / pre-tiled, bf16) ----
    xT = inp("xT", (C, 4, P, T))                    # [c][kk][p][t]  x^T slices
    xf = inp("xf", (C, 8, P, D), f32)               # [c][tm][p][d]  residual x
    wproj = {n: inp("w" + n, (4, P, 4, P)) for n in "qkv"}   # [kk][p][m][c] = W.T tiles
    bproj = {n: inp("b" + n, (P, 4), f32) for n in "qkv"}    # col m = bias of e-tile m
    w1 = {n: inp("w1" + n, (32, P, 2, 16, P)) for n in "qkv"}  # [i][p][ab][kk][c]
    b1 = {n: inp("b1" + n, (P, 64), f32) for n in "qkv"}
    w2 = {n: inp("w2" + n, (16, P, 2, 32, P)) for n in "qkv"}
    b2 = {n: inp("b2" + n, (P, 32), f32) for n in "qkv"}
    w3 = {n: inp("w3" + n, (16, P, 16, P)) for n in "qk"}      # [m][p][kk][c]
    b3 = {n: inp("b3" + n, (P, 16), f32) for n in "qk"}
    w3v = inp("w3v", (16, 2, P, 1024))              # [kk][nbp][p][c]
    b3v = inp("b3v", (1, A), f32)
    wo = inp("wo", (16, P, D))                      # [kk][p][d] = Wo.T tiles
    bo = inp("bo", (1, D), f32)
    lng4 = inp("lng4", (1, D), f32)                 # ln_g * 0.25
    lnb = inp("lnb", (1, D), f32)

    out = nc.declare_dram_parameter("out", [8, P, D], f32, isOutput=True)

    # ---- DRAM scratch ----
    qT_d = nc.dram_tensor("qT_d", [16, P, T], bf16)
    kT_d = nc.dram_tensor("kT_d", [16, P, T], bf16)
    vS_d = nc.dram_tensor("vS_d", [8, P, A], bf16)
    scratch = {"q": qT_d, "k": kT_d}

    with tile.TileContext(nc) as tc:
        import contextlib

        with contextlib.ExitStack() as stk:
            pc = stk.enter_context(tc.tile_pool(name="consts", bufs=1))
            pp = stk.enter_context(tc.tile_pool(name="ps", bufs=2, space="PSUM"))

            # constants
            ones_bf = pc.tile([P, P], bf16, tag="ones")
            nc.vector.memset(ones_bf[:], 1.0)
            eps_sb = pc.tile([P, 1], f32, tag="eps")
            nc.vector.memset(eps_sb[:], EPS)

            def bcast_load(src, width, tag):
                t = pc.tile([P, width], f32, tag=tag, name=tag)
                nc.sync.dma_start(out=t[:], in_=src[:].to_broadcast([P, width]))
                return t

            b3v_bc = bcast_load(b3v, A, "b3v_bc")
            bo_bc = bcast_load(bo, D, "bo_bc")
            lng4_bc = bcast_load(lng4, D, "lng4_bc")
            lnb_bc = bcast_load(lnb, D, "lnb_bc")

            bias_sb = {}
            for n in "qkv":
                for nm, hnd, w in (("p", bproj[n], 4), ("1", b1[n], 64),
                                   ("2", b2[n], 32)):
                    t = pc.tile([P, w], f32, tag=f"b{nm}{n}")
                    nc.sync.dma_start(out=t[:], in_=hnd[:])
                    bias_sb[nm + n] = t
            for n in "qk":
                t = pc.tile([P, 16], f32, tag=f"b3{n}")
                nc.sync.dma_start(out=t[:], in_=b3[n][:])
                bias_sb["3" + n] = t

            # =================== phase A: projections + gated chains =========
            with contextlib.ExitStack() as astk:
                p_xh2 = astk.enter_context(tc.tile_pool(name="xh2", bufs=17))
                p_in = astk.enter_context(tc.tile_pool(name="inev", bufs=17))
                p_h1 = astk.enter_context(tc.tile_pool(name="h1p", bufs=32))
                p_wb = astk.enter_context(tc.tile_pool(name="wbig", bufs=2))
                p_w3 = astk.enter_context(tc.tile_pool(name="w3p", bufs=2))
                p_sig = astk.enter_context(tc.tile_pool(name="sigp", bufs=2))

                for n in "qkv":
                    # ---- A1: in_proj -> inT (feature-major, 16 tiles) ----
                    wp_sb = p_w3.tile([P, 4, 4, P], bf16, tag="wproj", bufs=1)
                    for kk in range(4):
                        nc.sync.dma_start(out=wp_sb[:, kk], in_=wproj[n][kk])
                    xs = []
                    for c in range(C):
                        for kk in range(4):
                            t = p_xh2.tile([P, T], bf16, tag="xh2")
                            nc.sync.dma_start(out=t[:], in_=xT[c, kk])
                            xs.append(t)
                    inT = []
                    for c in range(C):
                        for m in range(4):
                            ps = pp.tile([P, T], f32, tag="pa")
                            for nh in range(2):
                                sl = slice(nh * 512, nh * 512 + 512)
                                for kk in range(4):
                                    nc.tensor.matmul(
                                        ps[:, sl], lhsT=wp_sb[:, kk, m, :],
                                        rhs=xs[c * 4 + kk][:, sl],
                                        start=(kk == 0), stop=(kk == 3))
                            t = p_in.tile([P, T], bf16, tag="inev")
                            nc.scalar.add(out=t[:], in_=ps[:],
                                          add=bias_sb["p" + n][:, m:m + 1])
                            inT.append(t)

                    # ---- A2: h1 = GLU(W1 @ inT + b1) (32 tiles) ----
                    h1 = []
                    for i in range(32):
                        wt = p_wb.tile([P, 2, 16, P], bf16, tag="wbig")
                        nc.sync.dma_start(out=wt[:], in_=w1[n][i])
                        pa = pp.tile([P, T], f32, tag="pa")
                        pb = pp.tile([P, T], f32, tag="pb")
                        for ab, ps in ((0, pa), (1, pb)):
                            for nh in range(2):
                                sl = slice(nh * 512, nh * 512 + 512)
                                for kk in range(16):
                                    nc.tensor.matmul(
                                        ps[:, sl], lhsT=wt[:, ab, kk, :],
                                        rhs=inT[kk][:, sl],
                                        start=(kk == 0), stop=(kk == 15))
                        sig = p_sig.tile([P, T], f32, tag="sig")
                        nc.scalar.activation(
                            out=sig[:], in_=pb[:], func=AF.Sigmoid,
                            bias=bias_sb["1" + n][:, 32 + i:33 + i], scale=1.0)
                        t = p_h1.tile([P, T], bf16, tag="h1")
                        nc.vector.scalar_tensor_tensor(
                            out=t[:], in0=pa[:],
                            scalar=bias_sb["1" + n][:, i:i + 1], in1=sig[:],
                            op0=OP.add, op1=OP.mult)
                        h1.append(t)

                    # ---- A3: h2 = GLU(W2 @ h1 + b2) (16 tiles) ----
                    h2 = []
                    for i in range(16):
                        wt = p_wb.tile([P, 2, 32, P], bf16, tag="wbig")
                        nc.sync.dma_start(out=wt[:], in_=w2[n][i])
                        pa = pp.tile([P, T], f32, tag="pa")
                        pb = pp.tile([P, T], f32, tag="pb")
                        for ab, ps in ((0, pa), (1, pb)):
                            for nh in range(2):
                                sl = slice(nh * 512, nh * 512 + 512)
                                for kk in range(32):
                                    nc.tensor.matmul(
                                        ps[:, sl], lhsT=wt[:, ab, kk, :],
                                        rhs=h1[kk][:, sl],
                                        start=(kk == 0), stop=(kk == 31))
                        sig = p_sig.tile([P, T], f32, tag="sig")
                        nc.scalar.activation(
                            out=sig[:], in_=pb[:], func=AF.Sigmoid,
                            bias=bias_sb["2" + n][:, 16 + i:17 + i], scale=1.0)
                        t = p_xh2.tile([P, T], bf16, tag="xh2")
                        nc.vector.scalar_tensor_tensor(
                            out=t[:], in0=pa[:],
                            scalar=bias_sb["2" + n][:, i:i + 1], in1=sig[:],
                            op0=OP.add, op1=OP.mult)
                        h2.append(t)

                    # ---- A4: last linear ----
                    if n in "qk":
                        # feature-major output -> q^T / k^T scratch
                        for m in range(16):
                            w3_sb = p_w3.tile([P, 16, P], bf16, tag="w3qk")
                            nc.sync.dma_start(out=w3_sb[:], in_=w3[n][m])
                            ps = pp.tile([P, T], f32, tag="pa")
                            for nh in range(2):
                                sl = slice(nh * 512, nh * 512 + 512)
                                for kk in range(16):
                                    nc.tensor.matmul(
                                        ps[:, sl], lhsT=w3_sb[:, kk, :],
                                        rhs=h2[kk][:, sl],
                                        start=(kk == 0), stop=(kk == 15))
                            t = p_in.tile([P, T], bf16, tag="inev")
                            nc.scalar.add(out=t[:], in_=ps[:],
                                          add=bias_sb["3" + n][:, m:m + 1])
                            nc.sync.dma_start(out=scratch[n][m], in_=t[:])
                    else:
                        # v: swap roles -> (t, feat) layout in vS_d
                        for nbp in range(2):
                            for mg in range(2):
                                pss = []
                                for j in range(4):
                                    pv = pp.tile([P, T], f32, name=f"pv{j}",
                                                 tag=("pa" if j % 2 == 0 else "pb"))
                                    pss.append(pv)
                                for kk in range(16):
                                    wv = p_w3.tile([P, 1024], bf16, tag="w3v")
                                    nc.sync.dma_start(out=wv[:], in_=w3v[kk, nbp])
                                    for j in range(4):
                                        m = mg * 4 + j
                                        for nh in range(2):
                                            sl = slice(nh * 512, nh * 512 + 512)
                                            nc.tensor.matmul(
                                                pss[j][:, sl],
                                                lhsT=h2[kk][:, m * P:(m + 1) * P],
                                                rhs=wv[:, sl],
                                                start=(kk == 0), stop=(kk == 15))
                                for j in range(4):
                                    t = p_in.tile([P, T], bf16, tag="inev")
                                    nc.vector.tensor_add(
                                        out=t[:], in0=pss[j][:],
                                        in1=b3v_bc[:, nbp * 1024:(nbp + 1) * 1024])
                                    nc.sync.dma_start(
                                        out=vS_d[mg * 4 + j, :,
                                                 nbp * 1024:(nbp + 1) * 1024],
                                        in_=t[:])

            # =================== phase B: attention ==========================
            with contextlib.ExitStack() as bstk:
                p_qk = bstk.enter_context(tc.tile_pool(name="qkp", bufs=8))
                p_exp = bstk.enter_context(tc.tile_pool(name="expp", bufs=10))
                p_vh = bstk.enter_context(tc.tile_pool(name="vhp", bufs=18))
                p_rec = bstk.enter_context(tc.tile_pool(name="recp", bufs=2))
                p_at = bstk.enter_context(tc.tile_pool(name="atp", bufs=16))
                attnT = []
                for h in range(H):
                    qs, ks_ = [], []
                    for j in range(2):
                        tq = p_qk.tile([P, T], bf16, tag="qk")
                        nc.sync.dma_start(out=tq[:], in_=qT_d[2 * h + j])
                        qs.append(tq)
                        tk = p_qk.tile([P, T], bf16, tag="qk")
                        nc.sync.dma_start(out=tk[:], in_=kT_d[2 * h + j])
                        ks_.append(tk)
                    vh = []
                    for sm in range(8):
                        tv = p_vh.tile([P, HD], bf16, tag="vh")
                        nc.sync.dma_start(out=tv[:],
                                          in_=vS_d[sm, :, h * HD:(h + 1) * HD])
                        vh.append(tv)
                    exps = []
                    for sm in range(8):
                        ps = pp.tile([P, T], f32, tag="pa")
                        for nh in range(2):
                            sl = slice(nh * 512, nh * 512 + 512)
                            for kk in range(2):
                                nc.tensor.matmul(
                                    ps[:, sl], lhsT=ks_[kk][:, sm * P:(sm + 1) * P],
                                    rhs=qs[kk][:, sl],
                                    start=(kk == 0), stop=(kk == 1))
                        e = p_exp.tile([P, T], bf16, tag="exp")
                        nc.scalar.activation(out=e[:], in_=ps[:], func=AF.Exp)
                        exps.append(e)
                    cs = pp.tile([P, T], f32, tag="pb")
                    for nh in range(2):
                        sl = slice(nh * 512, nh * 512 + 512)
                        for sm in range(8):
                            nc.tensor.matmul(
                                cs[:, sl], lhsT=ones_bf[:], rhs=exps[sm][:, sl],
                                start=(sm == 0), stop=(sm == 7))
                    rec = p_rec.tile([P, T], f32, tag="rec")
                    nc.vector.reciprocal(out=rec[:], in_=cs[:])
                    for um in range(2):
                        pu = pp.tile([P, T], f32, tag="pb")
                        for nh in range(2):
                            sl = slice(nh * 512, nh * 512 + 512)
                            for sm in range(8):
                                nc.tensor.matmul(
                                    pu[:, sl],
                                    lhsT=vh[sm][:, um * P:(um + 1) * P],
                                    rhs=exps[sm][:, sl],
                                    start=(sm == 0), stop=(sm == 7))
                        at = p_at.tile([P, T], bf16, tag="attnT")
                        nc.vector.tensor_mul(out=at[:], in0=pu[:], in1=rec[:])
                        attnT.append(at)

                # =============== phase C: out_proj + LN + mean over C ========
                with contextlib.ExitStack() as cstk:
                    p_wo = cstk.enter_context(tc.tile_pool(name="wop", bufs=17))
                    p_c = cstk.enter_context(tc.tile_pool(name="cp", bufs=4))
                    p_st = cstk.enter_context(tc.tile_pool(name="stp", bufs=6))
                    wo_sb = []
                    for kk in range(16):
                        t = p_wo.tile([P, D], bf16, tag="wo")
                        nc.sync.dma_start(out=t[:], in_=wo[kk])
                        wo_sb.append(t)
                    for tm in range(8):
                        po = pp.tile([P, T], f32, tag="pa")
                        for kk in range(16):
                            nc.tensor.matmul(
                                po[:, :D], lhsT=attnT[kk][:, tm * P:(tm + 1) * P],
                                rhs=wo_sb[kk][:],
                                start=(kk == 0), stop=(kk == 15))
                        ao = p_c.tile([P, D], f32, tag="ao")
                        nc.vector.tensor_add(out=ao[:], in0=po[:, :D], in1=bo_bc[:])
                        acc = p_c.tile([P, D], f32, tag="acc")
                        for c in range(C):
                            xt = p_c.tile([P, D], f32, tag="xc")
                            nc.sync.dma_start(out=xt[:], in_=xf[c, tm])
                            z = p_c.tile([P, D], f32, tag="z")
                            nc.vector.scalar_tensor_tensor(
                                out=z[:], in0=xt[:], scalar=2.0, in1=ao[:],
                                op0=OP.mult, op1=OP.add)
                            st = p_st.tile([P, 6], f32, tag="bn")
                            nc.vector.bn_stats(out=st[:], in_=z[:])
                            mv = p_st.tile([P, 2], f32, tag="mv")
                            nc.vector.bn_aggr(out=mv[:], in_=st[:])
                            std = p_st.tile([P, 1], f32, tag="sd")
                            nc.scalar.activation(out=std[:], in_=mv[:, 1:2],
                                                 func=AF.Sqrt, bias=eps_sb[:])
                            rstd = p_st.tile([P, 1], f32, tag="rs")
                            nc.vector.reciprocal(out=rstd[:], in_=std[:])
                            tgt = acc if c == 0 else p_c.tile([P, D], f32, tag="nm")
                            nc.vector.tensor_scalar(
                                out=tgt[:], in0=z[:], scalar1=mv[:, 0:1],
                                scalar2=rstd[:], op0=OP.subtract, op1=OP.mult)
                            if c > 0:
                                nc.vector.tensor_add(out=acc[:], in0=acc[:],
                                                     in1=tgt[:])
                        o = p_c.tile([P, D], f32, tag="oo")
                        nc.vector.tensor_mul(out=o[:], in0=acc[:], in1=lng4_bc[:])
                        nc.vector.tensor_add(out=o[:], in0=o[:], in1=lnb_bc[:])
                        nc.sync.dma_start(out=out[tm], in_=o[:])

    nc.compile()
    return nc


def _get_program():
    if "nc" not in _prog_cache:
        _prog_cache["nc"] = _build_program()
    return _prog_cache["nc"]


def _cbf(a):
    return np.ascontiguousarray(a).astype(BF)


def _prep_common(inputs):
    """Host-side weight re-tiling (shared across all cores)."""
    cm = {}
    sc = HD ** -0.5
    for n, (wk, bk_) in (("q", ("Wq", "bq")), ("k", ("Wk", "bk")),
                         ("v", ("Wv", "bv"))):
        Wt = np.asarray(inputs[wk], np.float32).T       # (D, E)
        cm["w" + n] = _cbf(Wt.reshape(4, P, 4, P))
        cm["b" + n] = np.ascontiguousarray(
            np.asarray(inputs[bk_], np.float32).reshape(4, P).T)
    for n, gk in (("q", "gq"), ("k", "gk"), ("v", "gv")):
        W1, b1, W2, b2, W3, b3 = [np.asarray(p, np.float32) for p in inputs[gk]]
        W1t = W1.T                                       # (2048, 8192)
        cm["w1" + n] = _cbf(W1t.reshape(16, P, 2, 32, P).transpose(3, 1, 2, 0, 4))
        cm["b1" + n] = np.ascontiguousarray(b1.reshape(64, P).T)
        W2t = W2.T                                       # (4096, 4096)
        cm["w2" + n] = _cbf(W2t.reshape(32, P, 2, 16, P).transpose(3, 1, 2, 0, 4))
        cm["b2" + n] = np.ascontiguousarray(b2.reshape(32, P).T)
        W3t = W3.T                                       # (2048, 2048)
        if n in "qk":
            s = sc if n == "q" else 1.0
            cm["w3" + n] = _cbf((W3t * s).reshape(16, P, 16, P).transpose(2, 1, 0, 3))
            cm["b3" + n] = np.ascontiguousarray((b3 * s).reshape(16, P).T)
        else:
            cm["w3v"] = _cbf(W3t.reshape(16, P, 2, 1024).transpose(0, 2, 1, 3))
            cm["b3v"] = np.ascontiguousarray(b3.reshape(1, A))
    cm["wo"] = _cbf(np.asarray(inputs["Wo"], np.float32).T.reshape(16, P, D))
    cm["bo"] = np.ascontiguousarray(np.asarray(inputs["bo"], np.float32).reshape(1, D))
    cm["lng4"] = np.ascontiguousarray(
        (np.asarray(inputs["ln_g"], np.float32) * 0.25).reshape(1, D))
    cm["lnb"] = np.ascontiguousarray(np.asarray(inputs["ln_b"], np.float32).reshape(1, D))
    return cm


def _run(inputs, trace=False):
    from concourse.bass_utils import run_bass_kernel_spmd

    nc = _get_program()
    cm = _prep_common(inputs)
    x = np.asarray(inputs["x"], np.float32)
    in_maps = []
    for b in range(B):
        m = dict(cm)
        m["xT"] = _cbf(x[b].transpose(0, 2, 1).reshape(C, 4, P, T))
        m["xf"] = np.ascontiguousarray(x[b].reshape(C, 8, P, D))
        in_maps.append(m)
    res = run_bass_kernel_spmd(nc, in_maps, core_ids=list(range(B)), trace=trace)
    out = np.stack([res.results[i]["out"].reshape(T, D) for i in range(B)])
    return out.astype(np.float32), res


def kernel(**inputs):
    out, _ = _run(inputs, trace=False)
    return out


# revision 7
# speedup vs baseline: 2.0436x; 2.0436x over previous
"""Trainium2 Bass kernel for nn_CoAttention_Simple (B=8,C=4,T=1024,D=512).

Sharding: data-parallel over B across the 8 NeuronCores (core i handles
batch b=i). Each core runs the full per-batch pipeline:

  in_proj (q/k/v) -> 3x gated-linear MLP -> 8-head attention ->
  out_proj -> double residual + LayerNorm -> mean over C

Phase A (projections + gated chains, ~93% of the FLOPs) runs in
fp8-e4m3 with DoubleRow perf mode (2 k-rows/partition/cycle); weights
are pre-scaled by 64 on the host so their ~0.02-magnitude values sit in
the fp8 normal range, and every PSUM eviction folds the 1/64 back in.
Attention and out_proj run in bf16. Softmax/layernorm math is fp32.

Activations flow feature-major ("transposed", (feat, t)) through the
gated chains so no on-device transposes are needed; the v-chain's last
matmul swaps lhsT/rhs roles to emit v in (t, feat) layout, which is
exactly what the attention AV matmul needs. Attention computes S^T=(s,t)
so the softmax denominator is a ones-matmul column sum (logits are ~1e-1
so exp needs no max subtraction).
"""

import sys

for _p in ("/opt/trn_rl_repo", "/root/.axon_site/_ro/trn_rl_repo"):
    if _p not in sys.path:
        sys.path.insert(0, _p)

import numpy as np
import ml_dtypes

B, C, T, D = 8, 4, 1024, 512
E = 512
A = C * E          # 2048
H = 8
HD = A // H        # 256
EPS = 1e-5
P = 128
BF = ml_dtypes.bfloat16
F8 = ml_dtypes.float8_e4m3
WS = 64.0          # fp8 weight pre-scale
WSI = 1.0 / WS

_prog_cache = {}


def _build_program():
    import concourse.mybir as mybir
    import concourse.tile as tile
    from concourse import bacc

    dt = mybir.dt
    f32, bf16, f8 = dt.float32, dt.bfloat16, dt.float8e4
    AF = mybir.ActivationFunctionType
    OP = mybir.AluOpType
    DR = mybir.MatmulPerfMode.DoubleRow

    nc = bacc.Bacc(None, target_bir_lowering=False)

    def inp(name, shape, dty=f8):
        return nc.declare_dram_parameter(name, list(shape), dty, isOutput=False)

    # ---- external inputs (host pre-transposed / pre-tiled) ----
    xT = inp("xT", (C, P, 4, T))                    # [c][p][kk][t]  x^T fp8
    xf = inp("xf", (C, 8, P, D), f32)               # [c][tm][p][d]  residual x
    wproj = {n: inp("w" + n, (4, P, 4, P)) for n in "qkv"}   # [kk][p][m][c]
    bproj = {n: inp("b" + n, (P, 4), f32) for n in "qkv"}
    w1 = {n: inp("w1" + n, (32, P, 2, 16, P)) for n in "qkv"}  # [i][p][ab][kk][c]
    b1 = {n: inp("b1" + n, (P, 64), f32) for n in "qkv"}
    w2 = {n: inp("w2" + n, (16, P, 2, 32, P)) for n in "qkv"}
    b2 = {n: inp("b2" + n, (P, 32), f32) for n in "qkv"}
    w3 = {n: inp("w3" + n, (16, P, 16, P)) for n in "qk"}      # [m][p][kk][c]
    b3 = {n: inp("b3" + n, (P, 16), f32) for n in "qk"}
    w3v = inp("w3v", (8, P, 2, 2, 1024))            # [kkp][p][j][nbp][c]
    b3v = inp("b3v", (1, A), f32)
    wo = inp("wo", (16, P, D), dt.bfloat16)         # [kk][p][d] = Wo.T tiles
    bo = inp("bo", (1, D), f32)
    lng4 = inp("lng4", (1, D), f32)                 # ln_g * 0.25
    lnb = inp("lnb", (1, D), f32)

    out = nc.declare_dram_parameter("out", [8, P, D], f32, isOutput=True)

    # ---- DRAM scratch ----
    qT_d = nc.dram_tensor("qT_d", [16, P, T], bf16)
    kT_d = nc.dram_tensor("kT_d", [16, P, T], bf16)
    vS_d = nc.dram_tensor("vS_d", [8, P, A], bf16)
    scratch = {"q": qT_d, "k": kT_d}

    with tile.TileContext(nc) as tc:
        import contextlib

        with contextlib.ExitStack() as stk:
            pc = stk.enter_context(tc.tile_pool(name="consts", bufs=1))
            pp = stk.enter_context(tc.tile_pool(name="ps", bufs=2, space="PSUM"))

            # constants
            ones_bf = pc.tile([P, P], bf16, tag="ones")
            nc.vector.memset(ones_bf[:], 1.0)
            eps_sb = pc.tile([P, 1], f32, tag="eps")
            nc.vector.memset(eps_sb[:], EPS)

            def bcast_load(src, width, tag):
                t = pc.tile([P, width], f32, tag=tag, name=tag)
                nc.sync.dma_start(out=t[:], in_=src[:].to_broadcast([P, width]))
                return t

            b3v_bc = bcast_load(b3v, A, "b3v_bc")
            bo_bc = bcast_load(bo, D, "bo_bc")
            lng4_bc = bcast_load(lng4, D, "lng4_bc")
            lnb_bc = bcast_load(lnb, D, "lnb_bc")

            bias_sb = {}
            for n in "qkv":
                for nm, hnd, w in (("p", bproj[n], 4), ("1", b1[n], 64),
                                   ("2", b2[n], 32)):
                    t = pc.tile([P, w], f32, tag=f"b{nm}{n}")
                    nc.sync.dma_start(out=t[:], in_=hnd[:])
                    bias_sb[nm + n] = t
            for n in "qk":
                t = pc.tile([P, 16], f32, tag=f"b3{n}")
                nc.sync.dma_start(out=t[:], in_=b3[n][:])
                bias_sb["3" + n] = t

            # =================== phase A: projections + gated chains (fp8) ===
            with contextlib.ExitStack() as astk:
                p_act = astk.enter_context(tc.tile_pool(name="acts", bufs=1))
                p_wb = astk.enter_context(tc.tile_pool(name="wbig", bufs=3))
                p_w3 = astk.enter_context(tc.tile_pool(name="w3p", bufs=3))
                p_ev = astk.enter_context(tc.tile_pool(name="evp", bufs=3))

                def glu_layer(nwide, i, wt, rhs_big, bias_t, out_big, nkp):
                    """one GLU output tile: out_big[:, i, :] (fp8)"""
                    pa = pp.tile([P, T], f32, tag="pa")
                    pb = pp.tile([P, T], f32, tag="pb")
                    for ab, ps in ((0, pa), (1, pb)):
                        for nh in range(2):
                            sl = slice(nh * 512, nh * 512 + 512)
                            for kp in range(nkp):
                                nc.tensor.matmul(
                                    ps[:, sl],
                                    lhsT=wt[:, ab, 2 * kp:2 * kp + 2, :],
                                    rhs=rhs_big[:, 2 * kp:2 * kp + 2, sl],
                                    start=(kp == 0), stop=(kp == nkp - 1),
                                    perf_mode=DR)
                    sig = p_ev.tile([P, T], f32, tag="sig", bufs=2)
                    nc.scalar.activation(
                        out=sig[:], in_=pb[:], func=AF.Sigmoid,
                        bias=bias_t[:, nwide + i:nwide + i + 1], scale=WSI)
                    ta = p_ev.tile([P, T], f32, tag="tmpa", bufs=2)
                    nc.scalar.activation(
                        out=ta[:], in_=pa[:], func=AF.Identity,
                        bias=bias_t[:, i:i + 1], scale=WSI)
                    nc.vector.tensor_mul(out=out_big[:, i, :], in0=ta[:],
                                         in1=sig[:])

                for n in "qkv":
                    # ---- A1: in_proj -> inT (feature-major fp8) ----
                    wp_sb = p_w3.tile([P, 4, 4, P], f8, tag="wproj", bufs=1)
                    for kk in range(4):
                        nc.sync.dma_start(out=wp_sb[:, kk], in_=wproj[n][kk])
                    xcs = []
                    for c in range(C):
                        xc = p_act.tile([P, 4, T], f8, tag="xt", bufs=5)
                        nc.sync.dma_start(out=xc[:], in_=xT[c])
                        xcs.append(xc)
                    inT = p_act.tile([P, 16, T], f8, tag="inT", bufs=2)
                    for c in range(C):
                        for m in range(4):
                            ps = pp.tile([P, T], f32, tag="pa")
                            for nh in range(2):
                                sl = slice(nh * 512, nh * 512 + 512)
                                for kp in range(2):
                                    nc.tensor.matmul(
                                        ps[:, sl],
                                        lhsT=wp_sb[:, 2 * kp:2 * kp + 2, m, :],
                                        rhs=xcs[c][:, 2 * kp:2 * kp + 2, sl],
                                        start=(kp == 0), stop=(kp == 1),
                                        perf_mode=DR)
                            nc.scalar.activation(
                                out=inT[:, c * 4 + m, :], in_=ps[:],
                                func=AF.Identity,
                                bias=bias_sb["p" + n][:, m:m + 1], scale=WSI)

                    # ---- A2: h1 = GLU(W1 @ inT + b1) ----
                    h1b = p_act.tile([P, 32, T], f8, tag="h1b", bufs=1)
                    for i in range(32):
                        wt = p_wb.tile([P, 2, 16, P], f8, tag="w1", bufs=3)
                        nc.sync.dma_start(out=wt[:], in_=w1[n][i])
                        glu_layer(32, i, wt, inT, bias_sb["1" + n], h1b, 8)

                    # ---- A3: h2 = GLU(W2 @ h1 + b2) ----
                    h2b = p_act.tile([P, 16, T], f8, tag="h2b", bufs=2)
                    for i in range(16):
                        wt = p_wb.tile([P, 2, 32, P], f8, tag="w2", bufs=2)
                        nc.sync.dma_start(out=wt[:], in_=w2[n][i])
                        glu_layer(16, i, wt, h1b, bias_sb["2" + n], h2b, 16)

                    # ---- A4: last linear ----
                    if n in "qk":
                        for m in range(16):
                            w3_sb = p_w3.tile([P, 16, P], f8, tag="w3qk", bufs=3)
                            nc.sync.dma_start(out=w3_sb[:], in_=w3[n][m])
                            ps = pp.tile([P, T], f32, tag="pa")
                            for nh in range(2):
                                sl = slice(nh * 512, nh * 512 + 512)
                                for kp in range(8):
                                    nc.tensor.matmul(
                                        ps[:, sl],
                                        lhsT=w3_sb[:, 2 * kp:2 * kp + 2, :],
                                        rhs=h2b[:, 2 * kp:2 * kp + 2, sl],
                                        start=(kp == 0), stop=(kp == 7),
                                        perf_mode=DR)
                            t = p_ev.tile([P, T], bf16, tag="ev", bufs=3)
                            nc.scalar.activation(
                                out=t[:], in_=ps[:], func=AF.Identity,
                                bias=bias_sb["3" + n][:, m:m + 1], scale=WSI)
                            nc.sync.dma_start(out=scratch[n][m], in_=t[:])
                    else:
                        # v: swap roles -> (t, feat) layout in vS_d
                        for nbp in range(2):
                            for mg in range(2):
                                pss = []
                                for j in range(4):
                                    pv = pp.tile([P, T], f32, name=f"pv{j}",
                                                 tag=("pa" if j % 2 == 0 else "pb"))
                                    pss.append(pv)
                                for kkp in range(8):
                                    wv = p_w3.tile([P, 2, 1024], f8, tag="w3v",
                                                   bufs=3)
                                    nc.sync.dma_start(
                                        out=wv[:], in_=w3v[kkp, :, :, nbp, :])
                                    for j in range(4):
                                        m = mg * 4 + j
                                        for nh in range(2):
                                            sl = slice(nh * 512, nh * 512 + 512)
                                            nc.tensor.matmul(
                                                pss[j][:, sl],
                                                lhsT=h2b[:, 2 * kkp:2 * kkp + 2,
                                                         m * P:(m + 1) * P],
                                                rhs=wv[:, :, sl],
                                                start=(kkp == 0), stop=(kkp == 7),
                                                perf_mode=DR)
                                for j in range(4):
                                    t = p_ev.tile([P, T], bf16, tag="ev", bufs=3)
                                    nc.vector.scalar_tensor_tensor(
                                        out=t[:], in0=pss[j][:], scalar=WSI,
                                        in1=b3v_bc[:, nbp * 1024:(nbp + 1) * 1024],
                                        op0=OP.mult, op1=OP.add)
                                    nc.sync.dma_start(
                                        out=vS_d[mg * 4 + j, :,
                                                 nbp * 1024:(nbp + 1) * 1024],
                                        in_=t[:])

            # =================== phase B: attention (bf16) ===================
            with contextlib.ExitStack() as bstk:
                p_qk = bstk.enter_context(tc.tile_pool(name="qkp", bufs=8))
                p_exp = bstk.enter_context(tc.tile_pool(name="expp", bufs=10))
                p_vh = bstk.enter_context(tc.tile_pool(name="vhp", bufs=18))
                p_rec = bstk.enter_context(tc.tile_pool(name="recp", bufs=2))
                p_at = bstk.enter_context(tc.tile_pool(name="atp", bufs=16))
                attnT = []
                for h in range(H):
                    qs, ks_ = [], []
                    for j in range(2):
                        tq = p_qk.tile([P, T], bf16, tag="qk")
                        nc.sync.dma_start(out=tq[:], in_=qT_d[2 * h + j])
                        qs.append(tq)
                        tk = p_qk.tile([P, T], bf16, tag="qk")
                        nc.sync.dma_start(out=tk[:], in_=kT_d[2 * h + j])
                        ks_.append(tk)
                    vh = []
                    for sm in range(8):
                        tv = p_vh.tile([P, HD], bf16, tag="vh")
                        nc.sync.dma_start(out=tv[:],
                                          in_=vS_d[sm, :, h * HD:(h + 1) * HD])
                        vh.append(tv)
                    exps = []
                    for sm in range(8):
                        ps = pp.tile([P, T], f32, tag="pa")
                        for nh in range(2):
                            sl = slice(nh * 512, nh * 512 + 512)
                            for kk in range(2):
                                nc.tensor.matmul(
                                    ps[:, sl], lhsT=ks_[kk][:, sm * P:(sm + 1) * P],
                                    rhs=qs[kk][:, sl],
                                    start=(kk == 0), stop=(kk == 1))
                        e = p_exp.tile([P, T], bf16, tag="exp")
                        nc.scalar.activation(out=e[:], in_=ps[:], func=AF.Exp)
                        exps.append(e)
                    cs = pp.tile([P, T], f32, tag="pb")
                    for nh in range(2):
                        sl = slice(nh * 512, nh * 512 + 512)
                        for sm in range(8):
                            nc.tensor.matmul(
                                cs[:, sl], lhsT=ones_bf[:], rhs=exps[sm][:, sl],
                                start=(sm == 0), stop=(sm == 7))
                    rec = p_rec.tile([P, T], f32, tag="rec")
                    nc.vector.reciprocal(out=rec[:], in_=cs[:])
                    for um in range(2):
                        pu = pp.tile([P, T], f32, tag="pb")
                        for nh in range(2):
                            sl = slice(nh * 512, nh * 512 + 512)
                            for sm in range(8):
                                nc.tensor.matmul(
                                    pu[:, sl],
                                    lhsT=vh[sm][:, um * P:(um + 1) * P],
                                    rhs=exps[sm][:, sl],
                                    start=(sm == 0), stop=(sm == 7))
                        at = p_at.tile([P, T], bf16, tag="attnT")
                        nc.vector.tensor_mul(out=at[:], in0=pu[:], in1=rec[:])
                        attnT.append(at)

                # =============== phase C: out_proj + LN + mean over C ========
                with contextlib.ExitStack() as cstk:
                    p_wo = cstk.enter_context(tc.tile_pool(name="wop", bufs=17))
                    p_c = cstk.enter_context(tc.tile_pool(name="cp", bufs=4))
                    p_st = cstk.enter_context(tc.tile_pool(name="stp", bufs=6))
                    wo_sb = []
                    for kk in range(16):
                        t = p_wo.tile([P, D], bf16, tag="wo")
                        nc.sync.dma_start(out=t[:], in_=wo[kk])
                        wo_sb.append(t)
                    for tm in range(8):
                        po = pp.tile([P, T], f32, tag="pa")
                        for kk in range(16):
                            nc.tensor.matmul(
                                po[:, :D], lhsT=attnT[kk][:, tm * P:(tm + 1) * P],
                                rhs=wo_sb[kk][:],
                                start=(kk == 0), stop=(kk == 15))
                        ao = p_c.tile([P, D], f32, tag="ao")
                        nc.vector.tensor_add(out=ao[:], in0=po[:, :D], in1=bo_bc[:])
                        acc = p_c.tile([P, D], f32, tag="acc")
                        for c in range(C):
                            xt = p_c.tile([P, D], f32, tag="xc")
                            nc.sync.dma_start(out=xt[:], in_=xf[c, tm])
                            z = p_c.tile([P, D], f32, tag="z")
                            nc.vector.scalar_tensor_tensor(
                                out=z[:], in0=xt[:], scalar=2.0, in1=ao[:],
                                op0=OP.mult, op1=OP.add)
                            st = p_st.tile([P, 6], f32, tag="bn")
                            nc.vector.bn_stats(out=st[:], in_=z[:])
                            mv = p_st.tile([P, 2], f32, tag="mv")
                            nc.vector.bn_aggr(out=mv[:], in_=st[:])
                            std = p_st.tile([P, 1], f32, tag="sd")
                            nc.scalar.activation(out=std[:], in_=mv[:, 1:2],
                                                 func=AF.Sqrt, bias=eps_sb[:])
                            rstd = p_st.tile([P, 1], f32, tag="rs")
                            nc.vector.reciprocal(out=rstd[:], in_=std[:])
                            tgt = acc if c == 0 else p_c.tile([P, D], f32, tag="nm")
                            nc.vector.tensor_scalar(
                                out=tgt[:], in0=z[:], scalar1=mv[:, 0:1],
                                scalar2=rstd[:], op0=OP.subtract, op1=OP.mult)
                            if c > 0:
                                nc.vector.tensor_add(out=acc[:], in0=acc[:],
                                                     in1=tgt[:])
                        o = p_c.tile([P, D], f32, tag="oo")
                        nc.vector.tensor_mul(out=o[:], in0=acc[:], in1=lng4_bc[:])
                        nc.vector.tensor_add(out=o[:], in0=o[:], in1=lnb_bc[:])
                        nc.sync.dma_start(out=out[tm], in_=o[:])

    nc.compile()
    return nc


def _get_program():
    if "nc" not in _prog_cache:
        _prog_cache["nc"] = _build_program()
    return _prog_cache["nc"]


def _c8(a):
    return np.ascontiguousarray(a).astype(F8)


def _prep_common(inputs):
    """Host-side weight re-tiling (shared across all cores)."""
    cm = {}
    sc = HD ** -0.5
    for n, (wk, bk_) in (("q", ("Wq", "bq")), ("k", ("Wk", "bk")),
                         ("v", ("Wv", "bv"))):
        Wt = np.asarray(inputs[wk], np.float32).T * WS      # (D, E)
        cm["w" + n] = _c8(Wt.reshape(4, P, 4, P))
        cm["b" + n] = np.ascontiguousarray(
            np.asarray(inputs[bk_], np.float32).reshape(4, P).T)
    for n, gk in (("q", "gq"), ("k", "gk"), ("v", "gv")):
        W1, b1, W2, b2, W3, b3 = [np.asarray(p, np.float32) for p in inputs[gk]]
        W1t = W1.T * WS                                     # (2048, 8192)
        cm["w1" + n] = _c8(W1t.reshape(16, P, 2, 32, P).transpose(3, 1, 2, 0, 4))
        cm["b1" + n] = np.ascontiguousarray(b1.reshape(64, P).T)
        W2t = W2.T * WS                                     # (4096, 4096)
        cm["w2" + n] = _c8(W2t.reshape(32, P, 2, 16, P).transpose(3, 1, 2, 0, 4))
        cm["b2" + n] = np.ascontiguousarray(b2.reshape(32, P).T)
        W3t = W3.T * WS                                     # (2048, 2048)
        if n in "qk":
            s = sc if n == "q" else 1.0
            cm["w3" + n] = _c8((W3t * s).reshape(16, P, 16, P).transpose(2, 1, 0, 3))
            cm["b3" + n] = np.ascontiguousarray((b3 * s).reshape(16, P).T)
        else:
            cm["w3v"] = _c8(W3t.reshape(8, 2, P, 2, 1024).transpose(0, 2, 1, 3, 4))
            cm["b3v"] = np.ascontiguousarray(b3.reshape(1, A))
    cm["wo"] = np.ascontiguousarray(
        np.asarray(inputs["Wo"], np.float32).T.reshape(16, P, D)).astype(BF)
    cm["bo"] = np.ascontiguousarray(np.asarray(inputs["bo"], np.float32).reshape(1, D))
    cm["lng4"] = np.ascontiguousarray(
        (np.asarray(inputs["ln_g"], np.float32) * 0.25).reshape(1, D))
    cm["lnb"] = np.ascontiguousarray(np.asarray(inputs["ln_b"], np.float32).reshape(1, D))
    return cm


def _run(inputs, trace=False):
    from concourse.bass_utils import run_bass_kernel_spmd

    nc = _get_program()
    cm = _prep_common(inputs)
    x = np.asarray(inputs["x"], np.float32)
    in_maps = []
    for b in range(B):
        m = dict(cm)
        # xT: [c][p][kk][t] with d = kk*128 + p
        m["xT"] = _c8(x[b].transpose(0, 2, 1).reshape(C, 4, P, T)
                      .transpose(0, 2, 1, 3))
        m["xf"] = np.ascontiguousarray(x[b].reshape(C, 8, P, D))
        in_maps.append(m)
    res = run_bass_kernel_spmd(nc, in_maps, core_ids=list(range(B)), trace=trace)
    out = np.stack([res.results[i]["out"].reshape(T, D) for i in range(B)])
    return out.astype(np.float32), res


def kernel(**inputs):
    out, _ = _run(inputs, trace=False)
    return out


# revision 9
# speedup vs baseline: 2.0444x; 1.0004x over previous
"""Trainium2 Bass kernel for nn_CoAttention_Simple (B=8,C=4,T=1024,D=512).

Sharding: data-parallel over B across the 8 NeuronCores (core i handles
batch b=i). Each core runs the full per-batch pipeline:

  in_proj (q/k/v) -> 3x gated-linear MLP -> 8-head attention ->
  out_proj -> double residual + LayerNorm -> mean over C

Phase A (projections + gated chains, ~93% of the FLOPs) runs in
fp8-e4m3 with DoubleRow perf mode (2 k-rows/partition/cycle); weights
are pre-scaled by 64 on the host so their ~0.02-magnitude values sit in
the fp8 normal range, and every PSUM eviction folds the 1/64 back in.
Attention and out_proj run in bf16. Softmax/layernorm math is fp32.

Activations flow feature-major ("transposed", (feat, t)) through the
gated chains so no on-device transposes are needed; the v-chain's last
matmul swaps lhsT/rhs roles to emit v in (t, feat) layout, which is
exactly what the attention AV matmul needs. Attention computes S^T=(s,t)
so the softmax denominator is a ones-matmul column sum (logits are ~1e-1
so exp needs no max subtraction).
"""

import sys

for _p in ("/opt/trn_rl_repo", "/root/.axon_site/_ro/trn_rl_repo"):
    if _p not in sys.path:
        sys.path.insert(0, _p)

import numpy as np
import ml_dtypes

B, C, T, D = 8, 4, 1024, 512
E = 512
A = C * E          # 2048
H = 8
HD = A // H        # 256
EPS = 1e-5
P = 128
BF = ml_dtypes.bfloat16
F8 = ml_dtypes.float8_e4m3
WS = 64.0          # fp8 weight pre-scale
WSI = 1.0 / WS

_prog_cache = {}


def _build_program():
    import concourse.mybir as mybir
    import concourse.tile as tile
    from concourse import bacc

    dt = mybir.dt
    f32, bf16, f8 = dt.float32, dt.bfloat16, dt.float8e4
    AF = mybir.ActivationFunctionType
    OP = mybir.AluOpType
    DR = mybir.MatmulPerfMode.DoubleRow

    nc = bacc.Bacc(None, target_bir_lowering=False)

    def inp(name, shape, dty=f8):
        return nc.declare_dram_parameter(name, list(shape), dty, isOutput=False)

    # ---- external inputs (host pre-transposed / pre-tiled) ----
    xT = inp("xT", (C, P, 4, T))                    # [c][p][kk][t]  x^T fp8
    xf = inp("xf", (C, 8, P, D), f32)               # [c][tm][p][d]  residual x
    wproj = {n: inp("w" + n, (4, P, 4, P)) for n in "qkv"}   # [kk][p][m][c]
    bproj = {n: inp("b" + n, (P, 4), f32) for n in "qkv"}
    w1 = {n: inp("w1" + n, (32, P, 2, 16, P)) for n in "qkv"}  # [i][p][ab][kk][c]
    b1 = {n: inp("b1" + n, (P, 64), f32) for n in "qkv"}
    w2 = {n: inp("w2" + n, (16, P, 2, 32, P)) for n in "qkv"}
    b2 = {n: inp("b2" + n, (P, 32), f32) for n in "qkv"}
    w3 = {n: inp("w3" + n, (16, P, 16, P)) for n in "qk"}      # [m][p][kk][c]
    b3 = {n: inp("b3" + n, (P, 16), f32) for n in "qk"}
    w3v = inp("w3v", (8, P, 2, 2, 1024))            # [kkp][p][j][nbp][c]
    b3v = inp("b3v", (1, A), f32)
    wo = inp("wo", (16, P, D), dt.bfloat16)         # [kk][p][d] = Wo.T tiles
    bo = inp("bo", (1, D), f32)
    lng4 = inp("lng4", (1, D), f32)                 # ln_g * 0.25
    lnb = inp("lnb", (1, D), f32)

    out = nc.declare_dram_parameter("out", [8, P, D], f32, isOutput=True)

    # ---- DRAM scratch ----
    qT_d = nc.dram_tensor("qT_d", [16, P, T], bf16)
    kT_d = nc.dram_tensor("kT_d", [16, P, T], bf16)
    vS_d = nc.dram_tensor("vS_d", [8, P, A], bf16)
    scratch = {"q": qT_d, "k": kT_d}

    with tile.TileContext(nc) as tc:
        import contextlib

        with contextlib.ExitStack() as stk:
            pc = stk.enter_context(tc.tile_pool(name="consts", bufs=1))
            pp = stk.enter_context(tc.tile_pool(name="ps", bufs=2, space="PSUM"))

            # constants
            ones_bf = pc.tile([P, P], bf16, tag="ones")
            nc.vector.memset(ones_bf[:], 1.0)
            eps_sb = pc.tile([P, 1], f32, tag="eps")
            nc.vector.memset(eps_sb[:], EPS)

            def bcast_load(src, width, tag):
                t = pc.tile([P, width], f32, tag=tag, name=tag)
                nc.sync.dma_start(out=t[:], in_=src[:].to_broadcast([P, width]))
                return t

            b3v_bc = bcast_load(b3v, A, "b3v_bc")
            bo_bc = bcast_load(bo, D, "bo_bc")
            lng4_bc = bcast_load(lng4, D, "lng4_bc")
            lnb_bc = bcast_load(lnb, D, "lnb_bc")

            bias_sb = {}
            for n in "qkv":
                for nm, hnd, w in (("p", bproj[n], 4), ("1", b1[n], 64),
                                   ("2", b2[n], 32)):
                    t = pc.tile([P, w], f32, tag=f"b{nm}{n}")
                    nc.sync.dma_start(out=t[:], in_=hnd[:])
                    bias_sb[nm + n] = t
            for n in "qk":
                t = pc.tile([P, 16], f32, tag=f"b3{n}")
                nc.sync.dma_start(out=t[:], in_=b3[n][:])
                bias_sb["3" + n] = t

            # =================== phase A: projections + gated chains (fp8) ===
            with contextlib.ExitStack() as astk:
                p_act = astk.enter_context(tc.tile_pool(name="acts", bufs=1))
                p_wb = astk.enter_context(tc.tile_pool(name="wbig", bufs=3))
                p_w3 = astk.enter_context(tc.tile_pool(name="w3p", bufs=3))
                p_ev = astk.enter_context(tc.tile_pool(name="evp", bufs=3))

                def glu_layer(nwide, i, wt, rhs_big, bias_t, out_big, nkp):
                    """one GLU output tile: out_big[:, i, :] (fp8)"""
                    pa = pp.tile([P, T], f32, tag="pa")
                    pb = pp.tile([P, T], f32, tag="pb")
                    for ab, ps in ((0, pa), (1, pb)):
                        for nh in range(2):
                            sl = slice(nh * 512, nh * 512 + 512)
                            for kp in range(nkp):
                                nc.tensor.matmul(
                                    ps[:, sl],
                                    lhsT=wt[:, ab, 2 * kp:2 * kp + 2, :],
                                    rhs=rhs_big[:, 2 * kp:2 * kp + 2, sl],
                                    start=(kp == 0), stop=(kp == nkp - 1),
                                    perf_mode=DR)
                    sig = p_ev.tile([P, T], f32, tag="sig", bufs=2)
                    nc.scalar.activation(
                        out=sig[:], in_=pb[:], func=AF.Sigmoid,
                        bias=bias_t[:, nwide + i:nwide + i + 1], scale=WSI)
                    ta = p_ev.tile([P, T], f32, tag="tmpa", bufs=2)
                    nc.scalar.activation(
                        out=ta[:], in_=pa[:], func=AF.Identity,
                        bias=bias_t[:, i:i + 1], scale=WSI)
                    nc.vector.tensor_mul(out=out_big[:, i, :], in0=ta[:],
                                         in1=sig[:])

                for n in "qkv":
                    # ---- A1: in_proj -> inT (feature-major fp8) ----
                    wp_sb = p_w3.tile([P, 4, 4, P], f8, tag="wproj", bufs=1)
                    for kk in range(4):
                        nc.sync.dma_start(out=wp_sb[:, kk], in_=wproj[n][kk])
                    xcs = []
                    for c in range(C):
                        xc = p_act.tile([P, 4, T], f8, tag="xt", bufs=5)
                        nc.sync.dma_start(out=xc[:], in_=xT[c])
                        xcs.append(xc)
                    inT = p_act.tile([P, 16, T], f8, tag="inT", bufs=2)
                    for c in range(C):
                        for m in range(4):
                            ps = pp.tile([P, T], f32, tag="pa")
                            for nh in range(2):
                                sl = slice(nh * 512, nh * 512 + 512)
                                for kp in range(2):
                                    nc.tensor.matmul(
                                        ps[:, sl],
                                        lhsT=wp_sb[:, 2 * kp:2 * kp + 2, m, :],
                                        rhs=xcs[c][:, 2 * kp:2 * kp + 2, sl],
                                        start=(kp == 0), stop=(kp == 1),
                                        perf_mode=DR)
                            nc.scalar.activation(
                                out=inT[:, c * 4 + m, :], in_=ps[:],
                                func=AF.Identity,
                                bias=bias_sb["p" + n][:, m:m + 1], scale=WSI)

                    # ---- A2: h1 = GLU(W1 @ inT + b1) ----
                    h1b = p_act.tile([P, 32, T], f8, tag="h1b", bufs=1)
                    for i in range(32):
                        wt = p_wb.tile([P, 2, 16, P], f8, tag="w1", bufs=3)
                        nc.sync.dma_start(out=wt[:], in_=w1[n][i])
                        glu_layer(32, i, wt, inT, bias_sb["1" + n], h1b, 8)

                    # ---- A3: h2 = GLU(W2 @ h1 + b2) ----
                    h2b = p_act.tile([P, 16, T], f8, tag="h2b", bufs=2)
                    for i in range(16):
                        wt = p_wb.tile([P, 2, 32, P], f8, tag="w2", bufs=2)
                        nc.sync.dma_start(out=wt[:], in_=w2[n][i])
                        glu_layer(16, i, wt, h1b, bias_sb["2" + n], h2b, 16)

                    # ---- A4: last linear ----
                    if n in "qk":
                        for m in range(16):
                            w3_sb = p_w3.tile([P, 16, P], f8, tag="w3qk", bufs=3)
                            nc.sync.dma_start(out=w3_sb[:], in_=w3[n][m])
                            ps = pp.tile([P, T], f32, tag="pa")
                            for nh in range(2):
                                sl = slice(nh * 512, nh * 512 + 512)
                                for kp in range(8):
                                    nc.tensor.matmul(
                                        ps[:, sl],
                                        lhsT=w3_sb[:, 2 * kp:2 * kp + 2, :],
                                        rhs=h2b[:, 2 * kp:2 * kp + 2, sl],
                                        start=(kp == 0), stop=(kp == 7),
                                        perf_mode=DR)
                            t = p_ev.tile([P, T], bf16, tag="ev", bufs=3)
                            nc.scalar.activation(
                                out=t[:], in_=ps[:], func=AF.Identity,
                                bias=bias_sb["3" + n][:, m:m + 1], scale=WSI)
                            nc.sync.dma_start(out=scratch[n][m], in_=t[:])
                    else:
                        # v: swap roles -> (t, feat) layout in vS_d
                        for nbp in range(2):
                            for mg in range(2):
                                pss = []
                                for j in range(4):
                                    pv = pp.tile([P, T], f32, name=f"pv{j}",
                                                 tag=("pa" if j % 2 == 0 else "pb"))
                                    pss.append(pv)
                                for kkp in range(8):
                                    wv = p_w3.tile([P, 2, 1024], f8, tag="w3v",
                                                   bufs=3)
                                    nc.sync.dma_start(
                                        out=wv[:], in_=w3v[kkp, :, :, nbp, :])
                                    for j in range(4):
                                        m = mg * 4 + j
                                        for nh in range(2):
                                            sl = slice(nh * 512, nh * 512 + 512)
                                            nc.tensor.matmul(
                                                pss[j][:, sl],
                                                lhsT=h2b[:, 2 * kkp:2 * kkp + 2,
                                                         m * P:(m + 1) * P],
                                                rhs=wv[:, :, sl],
                                                start=(kkp == 0), stop=(kkp == 7),
                                                perf_mode=DR)
                                for j in range(4):
                                    t = p_ev.tile([P, T], bf16, tag="ev", bufs=3)
                                    nc.vector.scalar_tensor_tensor(
                                        out=t[:], in0=pss[j][:], scalar=WSI,
                                        in1=b3v_bc[:, nbp * 1024:(nbp + 1) * 1024],
                                        op0=OP.mult, op1=OP.add)
                                    nc.sync.dma_start(
                                        out=vS_d[mg * 4 + j, :,
                                                 nbp * 1024:(nbp + 1) * 1024],
                                        in_=t[:])

            # =================== phase B: attention (bf16) ===================
            with contextlib.ExitStack() as bstk:
                p_qk = bstk.enter_context(tc.tile_pool(name="qkp", bufs=8))
                p_exp = bstk.enter_context(tc.tile_pool(name="expp", bufs=18))
                p_vh = bstk.enter_context(tc.tile_pool(name="vhp", bufs=18))
                p_rec = bstk.enter_context(tc.tile_pool(name="recp", bufs=2))
                p_at = bstk.enter_context(tc.tile_pool(name="atp", bufs=16))
                attnT = []
                for h in range(H):
                    qs, ks_ = [], []
                    for j in range(2):
                        tq = p_qk.tile([P, T], bf16, tag="qk")
                        nc.sync.dma_start(out=tq[:], in_=qT_d[2 * h + j])
                        qs.append(tq)
                        tk = p_qk.tile([P, T], bf16, tag="qk")
                        nc.sync.dma_start(out=tk[:], in_=kT_d[2 * h + j])
                        ks_.append(tk)
                    vh = []
                    for sm in range(8):
                        tv = p_vh.tile([P, HD], bf16, tag="vh")
                        nc.sync.dma_start(out=tv[:],
                                          in_=vS_d[sm, :, h * HD:(h + 1) * HD])
                        vh.append(tv)
                    exps = []
                    for sm in range(8):
                        ps = pp.tile([P, T], f32, tag="pa")
                        for nh in range(2):
                            sl = slice(nh * 512, nh * 512 + 512)
                            for kk in range(2):
                                nc.tensor.matmul(
                                    ps[:, sl], lhsT=ks_[kk][:, sm * P:(sm + 1) * P],
                                    rhs=qs[kk][:, sl],
                                    start=(kk == 0), stop=(kk == 1))
                        e = p_exp.tile([P, T], bf16, tag="exp")
                        nc.scalar.activation(out=e[:], in_=ps[:], func=AF.Exp)
                        exps.append(e)
                    cs = pp.tile([P, T], f32, tag="pb")
                    for nh in range(2):
                        sl = slice(nh * 512, nh * 512 + 512)
                        for sm in range(8):
                            nc.tensor.matmul(
                                cs[:, sl], lhsT=ones_bf[:], rhs=exps[sm][:, sl],
                                start=(sm == 0), stop=(sm == 7))
                    rec = p_rec.tile([P, T], f32, tag="rec")
                    nc.vector.reciprocal(out=rec[:], in_=cs[:])
                    for um in range(2):
                        pu = pp.tile([P, T], f32, tag="pb")
                        for nh in range(2):
                            sl = slice(nh * 512, nh * 512 + 512)
                            for sm in range(8):
                                nc.tensor.matmul(
                                    pu[:, sl],
                                    lhsT=vh[sm][:, um * P:(um + 1) * P],
                                    rhs=exps[sm][:, sl],
                                    start=(sm == 0), stop=(sm == 7))
                        at = p_at.tile([P, T], bf16, tag="attnT")
                        nc.vector.tensor_mul(out=at[:], in0=pu[:], in1=rec[:])
                        attnT.append(at)

                # =============== phase C: out_proj + LN + mean over C ========
                with contextlib.ExitStack() as cstk:
                    p_wo = cstk.enter_context(tc.tile_pool(name="wop", bufs=17))
                    p_c = cstk.enter_context(tc.tile_pool(name="cp", bufs=4))
                    p_st = cstk.enter_context(tc.tile_pool(name="stp", bufs=6))
                    wo_sb = []
                    for kk in range(16):
                        t = p_wo.tile([P, D], bf16, tag="wo")
                        nc.sync.dma_start(out=t[:], in_=wo[kk])
                        wo_sb.append(t)
                    for tm in range(8):
                        po = pp.tile([P, T], f32, tag="pa")
                        for kk in range(16):
                            nc.tensor.matmul(
                                po[:, :D], lhsT=attnT[kk][:, tm * P:(tm + 1) * P],
                                rhs=wo_sb[kk][:],
                                start=(kk == 0), stop=(kk == 15))
                        ao = p_c.tile([P, D], f32, tag="ao")
                        nc.vector.tensor_add(out=ao[:], in0=po[:, :D], in1=bo_bc[:])
                        # batched LN over the 4 c-slices: one ACT sqrt + one
                        # DVE reciprocal per tm instead of 4 cross-engine
                        # round-trips.
                        zs = []
                        mvs = p_st.tile([P, 4, 2], f32, tag="mvs", bufs=3)
                        for c in range(C):
                            xt = p_c.tile([P, D], f32, tag="xc", bufs=10)
                            nc.sync.dma_start(out=xt[:], in_=xf[c, tm])
                            z = p_c.tile([P, D], f32, tag="z", bufs=5)
                            nc.vector.scalar_tensor_tensor(
                                out=z[:], in0=xt[:], scalar=2.0, in1=ao[:],
                                op0=OP.mult, op1=OP.add)
                            st = p_st.tile([P, 6], f32, tag="bn")
                            nc.vector.bn_stats(out=st[:], in_=z[:])
                            nc.vector.bn_aggr(out=mvs[:, c, :], in_=st[:])
                            zs.append(z)
                        std4 = p_st.tile([P, 4], f32, tag="sd4", bufs=3)
                        nc.scalar.activation(out=std4[:], in_=mvs[:, :, 1],
                                             func=AF.Sqrt, bias=eps_sb[:])
                        rec4 = p_st.tile([P, 4], f32, tag="rc4", bufs=3)
                        nc.vector.reciprocal(out=rec4[:], in_=std4[:])
                        acc = p_c.tile([P, D], f32, tag="acc")
                        for c in range(C):
                            tgt = acc if c == 0 else p_c.tile([P, D], f32, tag="nm")
                            nc.vector.tensor_scalar(
                                out=tgt[:], in0=zs[c][:], scalar1=mvs[:, c, 0:1],
                                scalar2=rec4[:, c:c + 1],
                                op0=OP.subtract, op1=OP.mult)
                            if c > 0:
                                nc.vector.tensor_add(out=acc[:], in0=acc[:],
                                                     in1=tgt[:])
                        o = p_c.tile([P, D], f32, tag="oo")
                        nc.vector.tensor_mul(out=o[:], in0=acc[:], in1=lng4_bc[:])
                        nc.vector.tensor_add(out=o[:], in0=o[:], in1=lnb_bc[:])
                        nc.sync.dma_start(out=out[tm], in_=o[:])

    nc.compile()
    return nc


def _get_program():
    if "nc" not in _prog_cache:
        _prog_cache["nc"] = _build_program()
    return _prog_cache["nc"]


def _c8(a):
    return np.ascontiguousarray(a).astype(F8)


def _prep_common(inputs):
    """Host-side weight re-tiling (shared across all cores)."""
    cm = {}
    sc = HD ** -0.5
    for n, (wk, bk_) in (("q", ("Wq", "bq")), ("k", ("Wk", "bk")),
                         ("v", ("Wv", "bv"))):
        Wt = np.asarray(inputs[wk], np.float32).T * WS      # (D, E)
        cm["w" + n] = _c8(Wt.reshape(4, P, 4, P))
        cm["b" + n] = np.ascontiguousarray(
            np.asarray(inputs[bk_], np.float32).reshape(4, P).T)
    for n, gk in (("q", "gq"), ("k", "gk"), ("v", "gv")):
        W1, b1, W2, b2, W3, b3 = [np.asarray(p, np.float32) for p in inputs[gk]]
        W1t = W1.T * WS                                     # (2048, 8192)
        cm["w1" + n] = _c8(W1t.reshape(16, P, 2, 32, P).transpose(3, 1, 2, 0, 4))
        cm["b1" + n] = np.ascontiguousarray(b1.reshape(64, P).T)
        W2t = W2.T * WS                                     # (4096, 4096)
        cm["w2" + n] = _c8(W2t.reshape(32, P, 2, 16, P).transpose(3, 1, 2, 0, 4))
        cm["b2" + n] = np.ascontiguousarray(b2.reshape(32, P).T)
        W3t = W3.T * WS                                     # (2048, 2048)
        if n in "qk":
            s = sc if n == "q" else 1.0
            cm["w3" + n] = _c8((W3t * s).reshape(16, P, 16, P).transpose(2, 1, 0, 3))
            cm["b3" + n] = np.ascontiguousarray((b3 * s).reshape(16, P).T)
        else:
            cm["w3v"] = _c8(W3t.reshape(8, 2, P, 2, 1024).transpose(0, 2, 1, 3, 4))
            cm["b3v"] = np.ascontiguousarray(b3.reshape(1, A))
    cm["wo"] = np.ascontiguousarray(
        np.asarray(inputs["Wo"], np.float32).T.reshape(16, P, D)).astype(BF)
    cm["bo"] = np.ascontiguousarray(np.asarray(inputs["bo"], np.float32).reshape(1, D))
    cm["lng4"] = np.ascontiguousarray(
        (np.asarray(inputs["ln_g"], np.float32) * 0.25).reshape(1, D))
    cm["lnb"] = np.ascontiguousarray(np.asarray(inputs["ln_b"], np.float32).reshape(1, D))
    return cm


def _run(inputs, trace=False):
    from concourse.bass_utils import run_bass_kernel_spmd

    nc = _get_program()
    cm = _prep_common(inputs)
    x = np.asarray(inputs["x"], np.float32)
    in_maps = []
    for b in range(B):
        m = dict(cm)
        # xT: [c][p][kk][t] with d = kk*128 + p
        m["xT"] = _c8(x[b].transpose(0, 2, 1).reshape(C, 4, P, T)
                      .transpose(0, 2, 1, 3))
        m["xf"] = np.ascontiguousarray(x[b].reshape(C, 8, P, D))
        in_maps.append(m)
    res = run_bass_kernel_spmd(nc, in_maps, core_ids=list(range(B)), trace=trace)
    out = np.stack([res.results[i]["out"].reshape(T, D) for i in range(B)])
    return out.astype(np.float32), res


def kernel(**inputs):
    out, _ = _run(inputs, trace=False)
    return out
